# revision 7
# baseline (speedup 1.0000x reference)
"""Masked attention (B=4, M=N=4096, D=64) on 8 Trainium2 NeuronCores.

Sharding: batch (4) x m-halves (2) -> 8 cores, no cross-core communication.
Each core computes out[m, :] = softmax(mask(q@k^T)/sqrt(d)) @ v for its
2048 q rows against the full 4096 k/v rows of its batch.

v4 (see v3 docstring for the three-engine elementwise split A/B/F):
  - PV restructured over m-block PAIRS: the two PV matmuls of a chunk
    (m-blocks 2bp, 2bp+1) share one LDWEIGHTS of the vA chunk, restoring
    the weight-load amortization MH=512 had lost (PE cadence 215ns/MM
    instead of ~330).
  - q/k constant DMAs split into m-block/chunk-range pieces across the
    three DMA queues so the first QK waits on ~128KB, not 512KB.
  - keepalive trimmed to 5 matmuls (it only needs to bridge the DMA wait).
  - last two pairs are B,F so the scalar engine retires its final exp two
    pairs before the end and the DVE finishes the tail.
"""

import numpy as np
import ml_dtypes
from contextlib import ExitStack

import concourse.bacc as bacc
import concourse.mybir as mybir
import concourse.tile as tile
from concourse.bass_utils import run_bass_kernel_spmd

B, M, N, D = 4, 4096, 4096, 64
NCORES = 8
M_LOC = M // 2        # q rows per core
MH = 512              # m sub-block of one scores tile column-half
NMB = M_LOC // MH     # 4 m-blocks
NBP = NMB // 2        # 2 m-block-pairs
NCH = N // 128        # 32 n-chunks of 128
NPAIR = NCH // 2      # 16 chunk-pairs
SCALE = 1.0 / 8.0     # 1/sqrt(64)
EBIAS = -3.0
MASKC = 240.0         # fp8 mask subtractor: exp sees s/8 - 30 -> 0 in fp16
LOG2E = 1.4426950408889634
FE_A = SCALE * 1024.0 * LOG2E                  # fast-exp scale on raw s
FE_B = 1024.0 * (15.0 + EBIAS * LOG2E) - 44.0  # fp16-bits bias, centered
BF16 = mybir.dt.bfloat16
F32 = mybir.dt.float32
FP16 = mybir.dt.float16
FP8 = mybir.dt.float8e4
I16 = mybir.dt.int16

# pair flavor schedule (16 n-pairs, same at every m-block):
# A=exp+mult, B=maskadd+exp, F=fastexp+mult.
PAIR_TYPES = ["A", "F", "B", "A", "A", "B", "F", "A",
              "B", "A", "F", "A", "A", "A", "B", "F"]
assert len(PAIR_TYPES) == NPAIR

_NC = None
LAST_RESULTS = None   # BassKernelResults of the most recent run (for profiling)
TRACE = False
TRACE_KW = {}
_RUN_IDX = 0


def _build_nc():
    nc = bacc.Bacc("TRN2", target_bir_lowering=False, debug=False,
                   num_devices=NCORES)
    qT = nc.dram_tensor("qT", [128, M_LOC], FP16, kind="ExternalInput").ap()
    kT = nc.dram_tensor("kT", [128, NPAIR * 128], FP16,
                        kind="ExternalInput").ap()
    vA = nc.dram_tensor("vA", [128, NCH * (D + 1)], FP16,
                        kind="ExternalInput").ap()
    nmT = nc.dram_tensor("nmT", [N, M_LOC], FP16, kind="ExternalInput").ap()
    m8T = nc.dram_tensor("m8T", [N, M_LOC], FP8, kind="ExternalInput").ap()
    id8 = nc.dram_tensor("id8", [128, 128], FP8, kind="ExternalInput").ap()
    # raw accumulator output: out^T with the softmax denominator in row 64;
    # the host does the (tiny) divide + transpose during unsharding
    o = nc.dram_tensor("oT", [NBP, D + 1, 2 * MH], F32,
                       kind="ExternalOutput").ap()

    with tile.TileContext(nc) as tc, ExitStack() as ctx:
        const = ctx.enter_context(tc.tile_pool(name="const", bufs=1))
        m16pool = ctx.enter_context(tc.tile_pool(name="m16", bufs=6))
        m8pool = ctx.enter_context(tc.tile_pool(name="m8", bufs=4))
        epool = ctx.enter_context(tc.tile_pool(name="e", bufs=4))
        ppool = ctx.enter_context(tc.tile_pool(name="p", bufs=8))
        fpool = ctx.enter_context(tc.tile_pool(name="fin", bufs=2))
        spool = ctx.enter_context(tc.tile_pool(name="spsum", bufs=3, space="PSUM"))
        opool = ctx.enter_context(tc.tile_pool(name="opsum", bufs=1, space="PSUM"))

        # constants, split so the first QK waits on as little DMA as
        # possible; spread over the sync/scalar/gpsimd queues.
        kT_s = const.tile([128, NPAIR * 128], FP16)
        nc.scalar.dma_start(kT_s[:, 0:512], kT[:, 0:512])
        qT_s = const.tile([128, M_LOC], FP16)
        nc.sync.dma_start(qT_s[:, 0:MH], qT[:, 0:MH])
        nc.gpsimd.dma_start(qT_s[:, MH:2 * MH], qT[:, MH:2 * MH])
        nc.scalar.dma_start(kT_s[:, 512:NPAIR * 128], kT[:, 512:NPAIR * 128])
        nc.sync.dma_start(qT_s[:, 2 * MH:3 * MH], qT[:, 2 * MH:3 * MH])
        nc.gpsimd.dma_start(qT_s[:, 3 * MH:4 * MH], qT[:, 3 * MH:4 * MH])
        id8_s = const.tile([128, 128], FP8)
        nc.sync.dma_start(id8_s[:], id8)
        vA_s = const.tile([128, NCH * (D + 1)], FP16)
        nc.scalar.dma_start(vA_s[:], vA)
        ebias = const.tile([128, 1], F32)
        nc.vector.memset(ebias[:], EBIAS)
        # warmup operand with no DMA dependency (starts right after preamble)
        wsrc = const.tile([128, 512], BF16)
        nc.vector.memset(wsrc[:], 1.0)

        # a few dense K=128 matmuls bridge the PE from preamble-end to the
        # first QK (and start warming the HAM clock gate)
        wu = spool.tile([128, 2 * MH], F32, tag="s")
        for _ in range(5):
            nc.tensor.matmul(wu[:, 0:512], wsrc[:, 0:128], wsrc[:],
                             start=True, stop=True)

        for bp in range(NBP):
            o_ps = opool.tile([D + 1, 2 * MH], F32)
            pv_pending = []

            def flush_pv():
                # two PV matmuls per chunk (m-blocks 2bp, 2bp+1) sharing
                # one LDWEIGHTS of the vA chunk
                for ni, pr0, pr1 in pv_pending:
                    vch = vA_s[:, ni * (D + 1):(ni + 1) * (D + 1)]
                    nc.tensor.matmul(o_ps[:, 0:MH], vch, pr0,
                                     start=(ni == 0), stop=(ni == NCH - 1))
                    nc.tensor.matmul(o_ps[:, MH:2 * MH], vch, pr1,
                                     start=(ni == 0), stop=(ni == NCH - 1))
                pv_pending.clear()

            for pc in range(NPAIR):
                pt = PAIR_TYPES[pc]
                ni_e, ni_o = 2 * pc, 2 * pc + 1
                lhs_e = kT_s[0:64, pc * 128:(pc + 1) * 128]
                lhs_o = kT_s[64:128, pc * 128:(pc + 1) * 128]
                ps = []      # p tiles for sub-blocks 0,1
                for sub in range(2):
                    mb = 2 * bp + sub
                    rhs_e = qT_s[0:64, mb * MH:(mb + 1) * MH]
                    rhs_o = qT_s[64:128, mb * MH:(mb + 1) * MH]
                    S = spool.tile([128, 2 * MH], F32, tag="s")
                    qk_stop = pt != "B"
                    nc.tensor.matmul(S[:, 0:MH], lhs_e, rhs_e,
                                     start=True, stop=qk_stop,
                                     tile_position=(0, 0))
                    nc.tensor.matmul(S[:, MH:2 * MH], lhs_o, rhs_o,
                                     start=True, stop=qk_stop,
                                     tile_position=(64, 0))
                    if sub == 1:
                        # PV of the PREVIOUS pair between this pair's QKs
                        flush_pv()
                    if pt == "B":
                        m8 = m8pool.tile([128, 2 * MH], FP8)
                        m8_src = m8T[ni_e * 128:(ni_e + 2) * 128,
                                     mb * MH:(mb + 1) * MH].rearrange(
                                         "(t p) m -> p t m", t=2)
                        nc.gpsimd.dma_start(
                            m8[:].rearrange("p (t m) -> p t m", t=2), m8_src)
                        nc.tensor.matmul(S[:, 0:MH], id8_s[:], m8[:, 0:MH],
                                         start=False, stop=True)
                        nc.tensor.matmul(S[:, MH:2 * MH], id8_s[:],
                                         m8[:, MH:2 * MH],
                                         start=False, stop=True)
                        p = ppool.tile([128, 2 * MH], FP16)
                        nc.scalar.activation(p[:], S[:],
                                             mybir.ActivationFunctionType.Exp,
                                             bias=ebias[:], scale=SCALE)
                    else:
                        nm = m16pool.tile([128, 2 * MH], FP16)
                        nm_src = nmT[ni_e * 128:(ni_e + 2) * 128,
                                     mb * MH:(mb + 1) * MH].rearrange(
                                         "(t p) m -> p t m", t=2)
                        dmaq = nc.sync if (pc % 2 == 0) else nc.gpsimd
                        dmaq.dma_start(
                            nm[:].rearrange("p (t m) -> p t m", t=2), nm_src)
                        e = epool.tile([128, 2 * MH], FP16)
                        if pt == "A":
                            nc.scalar.activation(
                                e[:], S[:], mybir.ActivationFunctionType.Exp,
                                bias=ebias[:], scale=SCALE)
                        else:  # F: DVE fast-exp via int16 bitcast
                            nc.vector.tensor_scalar(
                                e[:].bitcast(I16), S[:], FE_A, FE_B,
                                mybir.AluOpType.mult, mybir.AluOpType.add)
                        p = ppool.tile([128, 2 * MH], FP16)
                        nc.vector.tensor_mul(p[:], e[:], nm[:])
                    ps.append(p)
                pv_pending.append((ni_e, ps[0][:, 0:MH], ps[1][:, 0:MH]))
                pv_pending.append((ni_o, ps[0][:, MH:2 * MH],
                                   ps[1][:, MH:2 * MH]))
            flush_pv()
            oT = fpool.tile([D + 1, 2 * MH], F32)
            nc.vector.tensor_copy(oT[:], o_ps[:])
            nc.sync.dma_start(o[bp], oT[:])
    nc.compile()
    return nc


def _get_nc():
    global _NC
    if _NC is None:
        _NC = _build_nc()
    return _NC


_ID8 = None


def _prep_core(q, k, v, mask, b, j):
    global _ID8
    qs = q[b, j * M_LOC:(j + 1) * M_LOC, :]
    qT = np.ascontiguousarray(qs.T).astype(np.float16)    # [64, 2048]
    qTp = np.concatenate([qT, qT], axis=0)                # [128, 2048]
    kTf = np.ascontiguousarray(k[b].T).astype(np.float16) # [64, 4096]
    kTp = np.empty((128, NPAIR * 128), np.float16)
    kTr = kTf.reshape(64, NCH, 128)
    kTp[0:64] = kTr[:, 0::2, :].reshape(64, -1)
    kTp[64:128] = kTr[:, 1::2, :].reshape(64, -1)
    vb = v[b]                                             # [4096, 64]
    vA = np.empty((128, NCH * (D + 1)), np.float16)
    vAr = vA.reshape(128, NCH, D + 1)
    vAr[:, :, :D] = vb.reshape(NCH, 128, D).transpose(1, 0, 2).astype(np.float16)
    vAr[:, :, D] = np.float16(1.0)
    mT = np.ascontiguousarray(mask[b, j * M_LOC:(j + 1) * M_LOC, :].T)
    nmT = (~mT).astype(np.float16)
    m8T = mT.astype(ml_dtypes.float8_e4m3)
    if _ID8 is None:
        _ID8 = (np.eye(128, dtype=np.float32) * -MASKC).astype(
            ml_dtypes.float8_e4m3)
    return {"qT": qTp, "kT": kTp, "vA": vA, "nmT": nmT, "m8T": m8T,
            "id8": _ID8}


def kernel(q, k, v, mask):
    global LAST_RESULTS, _RUN_IDX
    q = np.asarray(q, dtype=np.float32)
    k = np.asarray(k, dtype=np.float32)
    v = np.asarray(v, dtype=np.float32)
    mask = np.asarray(mask)
    nc = _get_nc()
    in_maps = [_prep_core(q, k, v, mask, c // 2, c % 2) for c in range(NCORES)]
    kw = dict(TRACE_KW)
    if "tmpdir" in kw:
        import os
        _RUN_IDX += 1
        kw["tmpdir"] = os.path.join(kw["tmpdir"], f"run{_RUN_IDX}")
        os.makedirs(kw["tmpdir"], exist_ok=True)
    res = run_bass_kernel_spmd(nc, in_maps, core_ids=list(range(NCORES)),
                               trace=TRACE, **kw)
    LAST_RESULTS = res
    out = np.empty((B, M, D), np.float32)
    for c in range(NCORES):
        b, j = divmod(c, 2)
        oT = res.results[c]["oT"]                      # [NBP, 65, 2*MH]
        for bp in range(NBP):
            for sub in range(2):
                blk = (oT[bp, :D, sub * MH:(sub + 1) * MH] /
                       oT[bp, D, sub * MH:(sub + 1) * MH])
                lo = j * M_LOC + (2 * bp + sub) * MH
                out[b, lo:lo + MH, :] = blk.T
    return out
